# revision 9
# baseline (speedup 1.0000x reference)
"""Trainium2 Bass kernel for nn_LocalInteractionsLayer.

Reference computation:
    seq_pairs [B=16, C=8, L=4096, 2] f32
    top = seq_pairs[..., 0]; bot = seq_pairs[..., 1]
    out[b, p, c*225 + i*15 + j] = top[b, c, p+i] * bot[b, c, p+j]
    for p in [0, P), i,j in [0, 15), P = L - 14 = 4082
    -> out [16, 4082, 1800] f32 (~470 MB; heavily output-write bound).

Strategy (v3, fp16 + position-chunked layout):
  - Data-parallel over batch: 2 batches per core on 8 cores.
  - Correctness gate is Frobenius rel-err < 2e-2; fp16 end-to-end costs
    ~4e-4, halving the dominant store traffic (58.8 -> 29.4 MB/core).
  - Each SBUF partition owns S=16 consecutive output positions: raw
    input span of S+14=30 values per channel (1.9x expansion instead of
    the 15x of a per-position window layout -> loads drop to ~1 MB/core)
    and stores become [128, S*1800] tiles whose per-partition rows are
    57.6 KB contiguous in HBM (measured ~1.5x faster than 3.6 KB rows).
  - Real-HW DVE operands allow at most 3 free AP dims (TENSOR3D), so the
    per-q top window is first TRIPLICATED on the otherwise-idle ActE
    (16 copies of 360 elems per tile); the DVE multiply then uses 15=5*3:
      out[p, q, (c,i,j1,j0)] = A3q[p, ((c,i),*,j0)] * Bw[p, (c,*,j1,j0)]
    so every DVE operand has an innermost unit-stride packed fp16 dim
    (2x_1p perf mode; measured ~0.35 ns/elem vs 0.8 for stride-0
    innermost) within 3 free dims.
"""

import sys

if "/opt/trn_rl_repo" not in sys.path:
    sys.path.insert(0, "/opt/trn_rl_repo")

import numpy as np
from numpy.lib.stride_tricks import sliding_window_view

import concourse.tile as tile
from concourse import bacc, mybir
from concourse.bass_utils import run_bass_kernel_spmd

W = 15             # window length (2*7+1)
WPAD = W - 1
B, C, L = 16, 8, 4096
P = L - WPAD       # 4082 valid output positions
FREE = C * W * W   # 1800
NCORES = 8
BPC = B // NCORES  # batches per core = 2
S = 16             # output positions per partition
SPAN = S + WPAD    # 30 raw values per channel per partition
TPOS = 128 * S     # 2048 positions per tile
NTILE = (P + TPOS - 1) // TPOS  # 2 tiles per batch
AW = C * SPAN      # 240 raw top values per partition per tile
BWW = C * SPAN     # 240 raw bot values
TW = AW + BWW      # 480 load elems per partition per tile
TRIP = C * W * 3   # 360: one q's triplicated top window (built on ActE)

_BUILD_CACHE: dict = {}


def _build(loop_iters: int = 1, load_eng: str = "scalar", store_mode: str = "sync",
           in_bufs: int = 2, out_bufs: int = 2):
    """Build + compile the per-core Bacc program (identical on all 8 cores)."""
    nc = bacc.Bacc("TRN2", target_bir_lowering=False, debug=False, num_devices=NCORES)
    dt = mybir.dt.float16

    inw_d = nc.dram_tensor("inw", [BPC, NTILE, 128, TW], dt, kind="ExternalInput")
    out_d = nc.dram_tensor("out", [BPC, P, FREE], dt, kind="ExternalOutput")

    ld = {"scalar": nc.scalar, "sync": nc.sync, "gpsimd": nc.gpsimd}[load_eng]

    with tile.TileContext(nc) as tc:
        with (
            tc.tile_pool(name="inp", bufs=in_bufs) as inp,
            tc.tile_pool(name="trp", bufs=4) as trp,
            tc.tile_pool(name="outp", bufs=out_bufs) as outp,
        ):
            def _store(bb, t, ot):
                """Store tile (b, t): positions t*TPOS + p*S + q."""
                base = t * TPOS
                npos = min(TPOS, P - base)          # 2048 or 2034
                nfull = npos // S                   # full partitions (128 or 127)
                dst = out_d[bb, base : base + nfull * S, :].rearrange(
                    "(p q) f -> p (q f)", q=S
                )
                if store_mode == "split":
                    h = nfull // 2
                    nc.sync.dma_start(dst[:h], ot[:h, :])
                    nc.scalar.dma_start(dst[h:nfull], ot[h:nfull, :])
                else:
                    nc.sync.dma_start(dst, ot[:nfull, :])
                rem = npos - nfull * S              # 0 or 2 tail positions
                if rem:
                    tdst = out_d[bb, base + nfull * S : base + npos, :].rearrange(
                        "(a q) f -> a (q f)", a=1
                    )
                    nc.sync.dma_start(tdst, ot[nfull : nfull + 1, : rem * FREE])

            def _body(_it=None):
                for bb in range(BPC):
                    for t in range(NTILE):
                        inwt = inp.tile([128, TW], dt, tag="inw")
                        ld.dma_start(inwt[:], inw_d[bb, t])
                        ot = outp.tile([128, S * FREE], dt, tag="ot")
                        a_all = inwt[:, :AW].rearrange("p (c x) -> p c x", c=C)
                        b_all = inwt[:, AW:].rearrange("p (c x) -> p c x", c=C)
                        for q in range(S):
                            # ActE triplicates this q's top window:
                            # a3[p, c*45 + i*3 + r] = top[c, q+i]
                            a3 = trp.tile([128, TRIP], dt, tag="a3")
                            nc.scalar.copy(
                                a3[:].rearrange("p (c i r) -> p c i r", c=C, i=W),
                                a_all[:, :, q : q + W]
                                .unsqueeze(3)
                                .broadcast_to((128, C, W, 3)),
                            )
                            a = (
                                a3[:]
                                .rearrange("p (c i r) -> p c i r", c=C, i=W)
                                .unsqueeze(3)
                                .broadcast_to((128, C, W, 5, 3))
                            )
                            bb_ap = (
                                b_all[:, :, q : q + W]
                                .rearrange("p c (j1 j0) -> p c j1 j0", j1=5)
                                .unsqueeze(2)
                                .broadcast_to((128, C, W, 5, 3))
                            )
                            o = ot[:, q * FREE : (q + 1) * FREE].rearrange(
                                "p (c i j1 j0) -> p c i j1 j0", c=C, i=W, j1=5
                            )
                            nc.vector.tensor_mul(o, a, bb_ap)
                        _store(bb, t, ot)

            if loop_iters == 1:
                _body()
            else:
                with tc.For_i(0, loop_iters, 1) as it:
                    _body(it)
    nc.compile()
    return nc


def _get_built(loop_iters: int = 1):
    nc = _BUILD_CACHE.get(loop_iters)
    if nc is None:
        nc = _build(loop_iters)
        _BUILD_CACHE[loop_iters] = nc
    return nc


def _prep(seq_pairs: np.ndarray) -> np.ndarray:
    """Host-side span extraction into the fp16 chunked device layout.

    inw[b, t, p, c*30 + x]       = top[b, c, t*2048 + p*16 + x]
    inw[b, t, p, 240 + c*30 + x] = bot[b, c, t*2048 + p*16 + x]
    (indices past L-1 read zero padding; those outputs are never stored).
    """
    sp = np.ascontiguousarray(seq_pairs, dtype=np.float32)
    padded = np.zeros((B, C, L + WPAD, 2), np.float16)
    padded[:, :, :L] = sp.astype(np.float16)
    sw = sliding_window_view(padded, SPAN, axis=2)  # [B, C, L-S+1, 2, SPAN]
    pos0 = (np.arange(NTILE)[:, None] * TPOS + np.arange(128)[None, :] * S)
    spans = sw[:, :, pos0]                          # [B, C, T, 128, 2, SPAN]
    spans = spans.transpose(0, 2, 3, 1, 5, 4)       # [B, T, 128, C, SPAN, 2]
    out = np.empty((B, NTILE, 128, TW), np.float16)
    out[..., :AW] = spans[..., 0].reshape(B, NTILE, 128, AW)
    out[..., AW:] = spans[..., 1].reshape(B, NTILE, 128, BWW)
    return out


def kernel(seq_pairs: np.ndarray) -> np.ndarray:
    assert tuple(np.shape(seq_pairs)) == (B, C, L, 2), (
        f"expected seq_pairs shape {(B, C, L, 2)}, got {np.shape(seq_pairs)}"
    )
    inw = _prep(seq_pairs)
    nc = _get_built()
    in_maps = [{"inw": inw[k * BPC : (k + 1) * BPC]} for k in range(NCORES)]
    last_err = None
    for _attempt in range(3):
        try:
            res = run_bass_kernel_spmd(nc, in_maps, list(range(NCORES))).results
            break
        except Exception as err:  # transient axon/PJRT hiccups — retry
            last_err = err
    else:
        raise last_err
    return np.concatenate(
        [res[k]["out"] for k in range(NCORES)], axis=0
    ).astype(np.float32)


# revision 12
# speedup vs baseline: 1.0312x; 1.0312x over previous
"""Trainium2 Bass kernel for nn_LocalInteractionsLayer.

Reference computation:
    seq_pairs [B=16, C=8, L=4096, 2] f32
    top = seq_pairs[..., 0]; bot = seq_pairs[..., 1]
    out[b, p, c*225 + i*15 + j] = top[b, c, p+i] * bot[b, c, p+j]
    for p in [0, P), i,j in [0, 15), P = L - 14 = 4082
    -> out [16, 4082, 1800] f32 (~470 MB; heavily output-write bound).

Strategy (v3, fp16 + position-chunked layout):
  - Data-parallel over batch: 2 batches per core on 8 cores.
  - Correctness gate is Frobenius rel-err < 2e-2; fp16 end-to-end costs
    ~4e-4, halving the dominant store traffic (58.8 -> 29.4 MB/core).
  - Each SBUF partition owns S=16 consecutive output positions: raw
    input span of S+14=30 values per channel (1.9x expansion instead of
    the 15x of a per-position window layout -> loads drop to ~1 MB/core)
    and stores become [128, S*1800] tiles whose per-partition rows are
    57.6 KB contiguous in HBM (measured ~1.5x faster than 3.6 KB rows).
  - Real-HW DVE operands allow at most 3 free AP dims (TENSOR3D) and run
    ~2-3x faster when every operand's innermost dim is unit-stride packed
    16-bit (measured 0.22-0.35 ns/elem vs 0.8 with a stride-0 innermost
    broadcast). So the DVE first expands the top spans 15x on-chip (one
    3600-elem copy per tile), after which each of the S=16 per-tile
    multiplies reads fully-packed APs:
      out[p, q, (c,i,j)] = A15[p, (c, q+i, j)] * Bw[p, (c, *, q+j)]
    (cross-engine ActE handoffs measured pathologically slow; keeping
    the expansion on the DVE costs ~3 us/tile and no semaphores).
"""

import sys

if "/opt/trn_rl_repo" not in sys.path:
    sys.path.insert(0, "/opt/trn_rl_repo")

import numpy as np
from numpy.lib.stride_tricks import sliding_window_view

import concourse.tile as tile
from concourse import bacc, mybir
from concourse.bass_utils import run_bass_kernel_spmd

W = 15             # window length (2*7+1)
WPAD = W - 1
B, C, L = 16, 8, 4096
P = L - WPAD       # 4082 valid output positions
FREE = C * W * W   # 1800
NCORES = 8
BPC = B // NCORES  # batches per core = 2
S = 16             # output positions per partition
SPAN = S + WPAD    # 30 raw values per channel per partition
TPOS = 128 * S     # 2048 positions per tile
NTILE = (P + TPOS - 1) // TPOS  # 2 tiles per batch
AW = C * SPAN      # 240 raw top values per partition per tile
BWW = C * SPAN     # 240 raw bot values
TW = AW + BWW      # 480 load elems per partition per tile
A15W = C * SPAN * W  # 3600: on-chip 15x-expanded top spans

_BUILD_CACHE: dict = {}


def _build(loop_iters: int = 1, load_eng: str = "scalar", store_mode: str = "sync",
           in_bufs: int = 2, out_bufs: int = 2):
    """Build + compile the per-core Bacc program (identical on all 8 cores)."""
    nc = bacc.Bacc("TRN2", target_bir_lowering=False, debug=False, num_devices=NCORES)
    dt = mybir.dt.float16

    inw_d = nc.dram_tensor("inw", [BPC, NTILE, 128, TW], dt, kind="ExternalInput")
    out_d = nc.dram_tensor("out", [BPC, P, FREE], dt, kind="ExternalOutput")

    ld = {"scalar": nc.scalar, "sync": nc.sync, "gpsimd": nc.gpsimd}[load_eng]

    with tile.TileContext(nc) as tc:
        with (
            tc.tile_pool(name="inp", bufs=in_bufs) as inp,
            tc.tile_pool(name="trp", bufs=4) as trp,
            tc.tile_pool(name="outp", bufs=out_bufs) as outp,
        ):
            def _store(bb, t, ot):
                """Store tile (b, t): positions t*TPOS + p*S + q."""
                base = t * TPOS
                npos = min(TPOS, P - base)          # 2048 or 2034
                nfull = npos // S                   # full partitions (128 or 127)
                dst = out_d[bb, base : base + nfull * S, :].rearrange(
                    "(p q) f -> p (q f)", q=S
                )
                if store_mode == "split":
                    h = nfull // 2
                    nc.sync.dma_start(dst[:h], ot[:h, :])
                    nc.scalar.dma_start(dst[h:nfull], ot[h:nfull, :])
                else:
                    nc.sync.dma_start(dst, ot[:nfull, :])
                rem = npos - nfull * S              # 0 or 2 tail positions
                if rem:
                    tdst = out_d[bb, base + nfull * S : base + npos, :].rearrange(
                        "(a q) f -> a (q f)", a=1
                    )
                    nc.sync.dma_start(tdst, ot[nfull : nfull + 1, : rem * FREE])

            def _body(_it=None):
                for bb in range(BPC):
                    for t in range(NTILE):
                        inwt = inp.tile([128, TW], dt, tag="inw")
                        ld.dma_start(inwt[:], inw_d[bb, t])
                        ot = outp.tile([128, S * FREE], dt, tag="ot")
                        a_all = inwt[:, :AW].rearrange("p (c x) -> p c x", c=C)
                        b_all = inwt[:, AW:].rearrange("p (c x) -> p c x", c=C)
                        # DVE expands the top spans 15x once per tile:
                        # a15[p, (c*30 + x)*15 + r] = top[c, x]. The muls can
                        # then read fully-packed [(450,8),(1,225)] APs (2
                        # free dims, unit stride) at the fast DVE rate.
                        a15 = trp.tile([128, A15W], dt, tag="a15")
                        a15v = a15[:].rearrange("p (c x r) -> p c x r", c=C, x=SPAN)
                        nc.vector.tensor_scalar_mul(
                            a15v,
                            a_all.unsqueeze(3).broadcast_to((128, C, SPAN, W)),
                            1.0,
                        )
                        for q in range(S):
                            a = a15v[:, :, q : q + W, :]  # [128, C, 15, 15]
                            bb_ap = (
                                b_all[:, :, q : q + W]
                                .unsqueeze(2)
                                .broadcast_to((128, C, W, W))
                            )
                            o = ot[:, q * FREE : (q + 1) * FREE].rearrange(
                                "p (c i j) -> p c i j", c=C, i=W
                            )
                            nc.vector.tensor_mul(o, a, bb_ap)
                        _store(bb, t, ot)

            if loop_iters == 1:
                _body()
            else:
                with tc.For_i(0, loop_iters, 1) as it:
                    _body(it)
    nc.compile()
    return nc


def _get_built(loop_iters: int = 1):
    nc = _BUILD_CACHE.get(loop_iters)
    if nc is None:
        nc = _build(loop_iters)
        _BUILD_CACHE[loop_iters] = nc
    return nc


def _prep(seq_pairs: np.ndarray) -> np.ndarray:
    """Host-side span extraction into the fp16 chunked device layout.

    inw[b, t, p, c*30 + x]       = top[b, c, t*2048 + p*16 + x]
    inw[b, t, p, 240 + c*30 + x] = bot[b, c, t*2048 + p*16 + x]
    (indices past L-1 read zero padding; those outputs are never stored).
    """
    sp = np.ascontiguousarray(seq_pairs, dtype=np.float32)
    padded = np.zeros((B, C, L + WPAD, 2), np.float16)
    padded[:, :, :L] = sp.astype(np.float16)
    sw = sliding_window_view(padded, SPAN, axis=2)  # [B, C, L-S+1, 2, SPAN]
    pos0 = (np.arange(NTILE)[:, None] * TPOS + np.arange(128)[None, :] * S)
    spans = sw[:, :, pos0]                          # [B, C, T, 128, 2, SPAN]
    spans = spans.transpose(0, 2, 3, 1, 5, 4)       # [B, T, 128, C, SPAN, 2]
    out = np.empty((B, NTILE, 128, TW), np.float16)
    out[..., :AW] = spans[..., 0].reshape(B, NTILE, 128, AW)
    out[..., AW:] = spans[..., 1].reshape(B, NTILE, 128, BWW)
    return out


def kernel(seq_pairs: np.ndarray) -> np.ndarray:
    assert tuple(np.shape(seq_pairs)) == (B, C, L, 2), (
        f"expected seq_pairs shape {(B, C, L, 2)}, got {np.shape(seq_pairs)}"
    )
    inw = _prep(seq_pairs)
    nc = _get_built()
    in_maps = [{"inw": inw[k * BPC : (k + 1) * BPC]} for k in range(NCORES)]
    last_err = None
    for _attempt in range(3):
        try:
            res = run_bass_kernel_spmd(nc, in_maps, list(range(NCORES))).results
            break
        except Exception as err:  # transient axon/PJRT hiccups — retry
            last_err = err
    else:
        raise last_err
    return np.concatenate(
        [res[k]["out"] for k in range(NCORES)], axis=0
    ).astype(np.float32)


# revision 15
# speedup vs baseline: 3.2680x; 3.1691x over previous
"""Trainium2 Bass kernel for nn_LocalInteractionsLayer.

Reference computation:
    seq_pairs [B=16, C=8, L=4096, 2] f32
    top = seq_pairs[..., 0]; bot = seq_pairs[..., 1]
    out[b, p, c*225 + i*15 + j] = top[b, c, p+i] * bot[b, c, p+j]
    for p in [0, P), i,j in [0, 15), P = L - 14 = 4082
    -> out [16, 4082, 1800] f32 (~470 MB; heavily output-write bound).

Strategy (v3, fp16 + position-chunked layout):
  - Data-parallel over batch: 2 batches per core on 8 cores.
  - Correctness gate is Frobenius rel-err < 2e-2; fp16 end-to-end costs
    ~4e-4, halving the dominant store traffic (58.8 -> 29.4 MB/core).
  - Each SBUF partition owns S=16 consecutive output positions: raw
    input span of S+14=30 values per channel (1.9x expansion instead of
    the 15x of a per-position window layout -> loads drop to ~1 MB/core)
    and stores become [128, S*1800] tiles whose per-partition rows are
    57.6 KB contiguous in HBM (measured ~1.5x faster than 3.6 KB rows).
  - Real-HW DVE operands allow at most 3 free AP dims (TENSOR3D) and run
    ~2-3x faster when every operand's innermost dim is unit-stride packed
    16-bit (measured 0.22-0.35 ns/elem vs 0.8 with a stride-0 innermost
    broadcast). So the DVE first expands the top spans 15x on-chip (one
    3600-elem copy per tile), after which each of the S=16 per-tile
    multiplies reads fully-packed APs:
      out[p, q, (c,i,j)] = A15[p, (c, q+i, j)] * Bw[p, (c, *, q+j)]
    (cross-engine ActE handoffs measured pathologically slow; keeping
    the expansion on the DVE costs ~3 us/tile and no semaphores).
"""

import sys

if "/opt/trn_rl_repo" not in sys.path:
    sys.path.insert(0, "/opt/trn_rl_repo")

import numpy as np
from numpy.lib.stride_tricks import sliding_window_view

import concourse.tile as tile
from concourse import bacc, mybir
from concourse.bass_utils import run_bass_kernel_spmd

W = 15             # window length (2*7+1)
WPAD = W - 1
B, C, L = 16, 8, 4096
P = L - WPAD       # 4082 valid output positions
FREE = C * W * W   # 1800
NCORES = 8
BPC = B // NCORES  # batches per core = 2
S = 16             # output positions per partition
SPAN = S + WPAD    # 30 raw values per channel per partition
TPOS = 128 * S     # 2048 positions per tile
NTILE = (P + TPOS - 1) // TPOS  # 2 tiles per batch
AW = C * SPAN      # 240 raw top values per partition per tile
BWW = C * SPAN     # 240 raw bot values
TW = AW + BWW      # 480 load elems per partition per tile
A15W = C * SPAN * W  # 3600: on-chip 15x-expanded top spans

_BUILD_CACHE: dict = {}


def _build(loop_iters: int = 1, load_eng: str = "scalar", store_mode: str = "sync",
           in_bufs: int = 2, out_bufs: int = 2, debug_mode: str = "full"):
    """Build + compile the per-core Bacc program (identical on all 8 cores)."""
    nc = bacc.Bacc("TRN2", target_bir_lowering=False, debug=False, num_devices=NCORES)
    dt = mybir.dt.float16

    inw_d = nc.dram_tensor("inw", [BPC, NTILE, 128, TW], dt, kind="ExternalInput")
    out_d = nc.dram_tensor("out", [BPC, P, FREE], dt, kind="ExternalOutput")

    ld = {"scalar": nc.scalar, "sync": nc.sync, "gpsimd": nc.gpsimd}[load_eng]

    with tile.TileContext(nc) as tc:
        with (
            tc.tile_pool(name="inp", bufs=in_bufs) as inp,
            tc.tile_pool(name="trp", bufs=4) as trp,
            tc.tile_pool(name="outp", bufs=out_bufs) as outp,
        ):
            def _store(bb, t, ot, store_gp=16):
                """Store tile (b, t): positions t*TPOS + p*S + q.

                Split into independent per-partition-group DMAs across both
                HWDGE rings: a single big DMA instruction only reaches ~50
                GB/s (one queue's worth); saturating the DMA engines needs
                several store instructions in flight.
                """
                base = t * TPOS
                npos = min(TPOS, P - base)          # 2048 or 2034
                nfull = npos // S                   # full partitions (128 or 127)
                dst = out_d[bb, base : base + nfull * S, :].rearrange(
                    "(p q) f -> p (q f)", q=S
                )
                for g0 in range(0, nfull, store_gp):
                    g1 = min(g0 + store_gp, nfull)
                    eng = nc.sync if (g0 // store_gp) % 2 == 0 else nc.scalar
                    eng.dma_start(dst[g0:g1], ot[g0:g1, :])
                rem = npos - nfull * S              # 0 or 2 tail positions
                if rem:
                    tdst = out_d[bb, base + nfull * S : base + npos, :].rearrange(
                        "(a q) f -> a (q f)", a=1
                    )
                    nc.sync.dma_start(tdst, ot[nfull : nfull + 1, : rem * FREE])

            def _body(_it=None):
                for bb in range(BPC):
                    for t in range(NTILE):
                        inwt = inp.tile([128, TW], dt, tag="inw")
                        ld.dma_start(inwt[:], inw_d[bb, t])
                        ot = outp.tile([128, S * FREE], dt, tag="ot")
                        a_all = inwt[:, :AW].rearrange("p (c x) -> p c x", c=C)
                        b_all = inwt[:, AW:].rearrange("p (c x) -> p c x", c=C)
                        # DVE expands the top spans 15x once per tile:
                        # a15[p, (c*30 + x)*15 + r] = top[c, x]. The muls can
                        # then read fully-packed [(450,8),(1,225)] APs (2
                        # free dims, unit stride) at the fast DVE rate.
                        a15 = trp.tile([128, A15W], dt, tag="a15")
                        a15v = a15[:].rearrange("p (c x r) -> p c x r", c=C, x=SPAN)
                        if debug_mode != "nomul":
                            nc.vector.tensor_scalar_mul(
                                a15v,
                                a_all.unsqueeze(3).broadcast_to((128, C, SPAN, W)),
                                1.0,
                            )
                            for q in range(S):
                                a = a15v[:, :, q : q + W, :]  # [128, C, 15, 15]
                                bb_ap = (
                                    b_all[:, :, q : q + W]
                                    .unsqueeze(2)
                                    .broadcast_to((128, C, W, W))
                                )
                                o = ot[:, q * FREE : (q + 1) * FREE].rearrange(
                                    "p (c i j) -> p c i j", c=C, i=W
                                )
                                nc.vector.tensor_mul(o, a, bb_ap)
                        else:
                            nc.vector.tensor_scalar_mul(
                                ot[:, :TW], inwt[:], 1.0)
                        if debug_mode != "nostore":
                            _store(bb, t, ot)

            if loop_iters == 1:
                _body()
            else:
                with tc.For_i(0, loop_iters, 1) as it:
                    _body(it)
    nc.compile()
    return nc


def _get_built(loop_iters: int = 1):
    nc = _BUILD_CACHE.get(loop_iters)
    if nc is None:
        nc = _build(loop_iters)
        _BUILD_CACHE[loop_iters] = nc
    return nc


def _prep(seq_pairs: np.ndarray) -> np.ndarray:
    """Host-side span extraction into the fp16 chunked device layout.

    inw[b, t, p, c*30 + x]       = top[b, c, t*2048 + p*16 + x]
    inw[b, t, p, 240 + c*30 + x] = bot[b, c, t*2048 + p*16 + x]
    (indices past L-1 read zero padding; those outputs are never stored).
    """
    sp = np.ascontiguousarray(seq_pairs, dtype=np.float32)
    padded = np.zeros((B, C, L + WPAD, 2), np.float16)
    padded[:, :, :L] = sp.astype(np.float16)
    sw = sliding_window_view(padded, SPAN, axis=2)  # [B, C, L-S+1, 2, SPAN]
    pos0 = (np.arange(NTILE)[:, None] * TPOS + np.arange(128)[None, :] * S)
    spans = sw[:, :, pos0]                          # [B, C, T, 128, 2, SPAN]
    spans = spans.transpose(0, 2, 3, 1, 5, 4)       # [B, T, 128, C, SPAN, 2]
    out = np.empty((B, NTILE, 128, TW), np.float16)
    out[..., :AW] = spans[..., 0].reshape(B, NTILE, 128, AW)
    out[..., AW:] = spans[..., 1].reshape(B, NTILE, 128, BWW)
    return out


def kernel(seq_pairs: np.ndarray) -> np.ndarray:
    assert tuple(np.shape(seq_pairs)) == (B, C, L, 2), (
        f"expected seq_pairs shape {(B, C, L, 2)}, got {np.shape(seq_pairs)}"
    )
    inw = _prep(seq_pairs)
    nc = _get_built()
    in_maps = [{"inw": inw[k * BPC : (k + 1) * BPC]} for k in range(NCORES)]
    last_err = None
    for _attempt in range(3):
        try:
            res = run_bass_kernel_spmd(nc, in_maps, list(range(NCORES))).results
            break
        except Exception as err:  # transient axon/PJRT hiccups — retry
            last_err = err
    else:
        raise last_err
    return np.concatenate(
        [res[k]["out"] for k in range(NCORES)], axis=0
    ).astype(np.float32)
